# revision 39
# baseline (speedup 1.0000x reference)
"""Trainium2 Bass kernel for nn_CausalPropagationAdjacency (v2).

Shapes (hardcoded): B=4, T=12, N=512, D=128, L=4, H=64.
Pipeline: lag encoders (Linear D->H, ReLU, Linear H->D, mean over L lags),
pairwise scorer sigmoid(relu(src_i+tgt_j+bs1)@Ws2+bs2), threshold 0.1, zero
diagonal, enhanced = A + 0.5 A^2 + 0.25 A^3, normalize by per-batch max.

Sharding: 8 cores = 4 batch-pairs. Each core scores 320 source rows: its
own 256 plus a REDUNDANT copy of the 64 rows of the peer's 4th group, so
only ONE AllGather (of each core's first 192 rows) is needed and its
~12-30us latency hides under the scoring of groups 3-4. Gather output is
rank-ordered == global row order, so every core assembles the full
(512,512) adjacency identically. Pure SPMD: the only per-core input
difference is xsrcP (the core's own 192 rows' lag slices); the two shared
64-row blocks (global rows 192:256 and 448:512) are projected from the
full target encoding with compile-time slices on BOTH cores. Host reads
core 2b's output.

Scoring: per source row one DVE tensor_scalar (add+max0) or ACT
Relu-with-bias produces relu(src_i+tgt+bs1) as a (128,512) bf16 tile; a
matmul against a 64-wide sliding window of the packed weight buffer (w2
embedded in one column) accumulates row i%64 of a (64,512) score block in
PSUM at the full 216ns/row streaming rate. 5 groups of 64 rows; groups 0-2
are sent (sigmoid->bf16->DMA->AllGather), groups 3/4 fill the two locally
known 64-row blocks via SBUF->SBUF DMA (partition shift). Threshold +
diagonal-zero run once per assembled A k-tile (off the scoring critical
path; gpsimd affine_select). Hops (transposes, A^2, A@a2,
identity-accumulated E) all in bf16 (fp32 PSUM accumulate). Global max:
per-partition max -> PE fp32 transpose -> one-partition reduce -> ones
matmul broadcast (avoids the gpsimd partition_all_reduce ucode-library
swap, ~10us). PE is pre-warmed with junk matmuls during the input-DMA
wait and between encoder and scoring so HAM stays at K=8/8; dummy ACT ops
preload both activation table sets off the critical path.
"""

import sys
import types
import numpy as np
import ml_dtypes

import concourse.bacc as bacc
import concourse.bass as bass
import concourse.bass_isa as bass_isa
import concourse.mybir as mybir
import concourse.tile as tile
from concourse.bass_utils import run_bass_kernel_spmd

B, T, N, D = 4, 12, 512, 128
L, H = 4, 64
THRESH = 0.1
NCORES = 8
NS = 320          # source slots per core: own 192 + block@192 + block@448
NSEND = 192       # rows sent through the AllGather
NT = N // 128
F32 = mybir.dt.float32
BF16 = mybir.dt.bfloat16
AF = mybir.ActivationFunctionType
ALU = mybir.AluOpType

# pairwise engine assignment per p%16 (DVE ~330-350ns/tile, ACT ~700ns/tile)
ACT_POS = {1, 4, 7, 10, 13}

WCOL = 639        # absolute wpk column holding Ws2 (inside the zero window)


def _build_nc():
    nc = bacc.Bacc("TRN2", target_bir_lowering=False, debug=False,
                   num_devices=NCORES)
    # host pre-transposed to (D, L*n) so the input DMAs are contiguous
    xlagT = nc.dram_tensor("xlagT", [D, L * N], BF16, kind="ExternalInput")
    # this core's own first 192 rows' lag slices (the per-core part)
    xsrcP = nc.dram_tensor("xsrcP", [D, L * NSEND], BF16,
                           kind="ExternalInput")
    # packed bf16 weights: [w1r(L*H=256) | ws1s(128) | ws1t(128) | zwin(255,
    #   w2 at abs col 639) | idbf(128) | 0.5*idbf(128) | pad(1)
    #   | bitcast f32 [bmean|bs1|bs2] (6) | pad(2)]
    wpk = nc.dram_tensor("wpk", [128, 1544], BF16, kind="ExternalInput")
    # w1 duplicated as its own small tensor so the encoder's first matmuls
    # depend on a 64KB DMA, not the full 386KB wpk transfer
    w1t = nc.dram_tensor("w1t", [128, 256], BF16, kind="ExternalInput")
    # w2r (64, L*D) bf16 + b1 (64, L) f32 bitcast to 2*L bf16 cols
    w2r = nc.dram_tensor("w2r", [H, L * D + 2 * L], BF16,
                         kind="ExternalInput")
    # bf16 output (host upcasts): halves the final DMA and speeds the
    # PSUM-read scale ops; ~4e-3 extra error vs the 2e-2 gate
    outfull = nc.dram_tensor("outfull", [N, N], BF16, kind="ExternalOutput")

    with tile.TileContext(nc) as tc:
        _emit(nc, tc, xlagT, xsrcP, wpk, w1t, w2r, outfull)
    nc.compile()
    return nc


def _emit(nc, tc, xlagT, xsrcP, wpk, w1t, w2r, outfull):
    from contextlib import ExitStack
    ctx = ExitStack()
    with ctx:
        consts = ctx.enter_context(tc.tile_pool(name="consts", bufs=1))
        sb = ctx.enter_context(tc.tile_pool(name="sb", bufs=1))
        relup = ctx.enter_context(tc.tile_pool(name="relu", bufs=10))
        workp = ctx.enter_context(tc.tile_pool(name="work", bufs=4))
        psA = ctx.enter_context(tc.tile_pool(name="psA", bufs=2, space="PSUM"))
        psB = ctx.enter_context(tc.tile_pool(name="psB", bufs=2, space="PSUM"))
        psE = ctx.enter_context(tc.tile_pool(name="psE", bufs=4, space="PSUM"))
        dram = ctx.enter_context(tc.tile_pool(name="dram", bufs=1,
                                              space="DRAM"))

        # ---- zero tile for PE pre-warm (no input dependency) ----
        z128 = consts.tile([128, 256], BF16, tag="z128")
        nc.gpsimd.memset(z128[:], 0.0)


        # ---- input DMAs: sync queue (small first); xlagT on scalar queue ----
        xfull_ = consts.tile([D, L * N], BF16, tag="xf")
        nc.scalar.dma_start(xfull_[:], xlagT[:])
        w1c = consts.tile([128, 256], BF16, tag="w1c")
        nc.sync.dma_start(w1c[:], w1t[:])
        xsrc_ = consts.tile([D, L * NSEND], BF16, tag="xs")
        nc.sync.dma_start(xsrc_[:], xsrcP[:])
        w2pk = consts.tile([H, L * D + 2 * L], BF16, tag="w2")
        nc.sync.dma_start(w2pk[:], w2r[:])
        wpks = consts.tile([128, 1544], BF16, tag="wpk")
        nc.sync.dma_start(wpks[:], wpk[:])
        xfull = xfull_.rearrange("d (l n) -> d l n", l=L)
        xsrc = xsrc_.rearrange("d (l n) -> d l n", l=L)

        w2sb = w2pk[:, 0:L * D].rearrange("h (l d) -> h l d", l=L)
        b1sb = w2pk[:, L * D:L * D + 2 * L].bitcast(F32)
        w1sb = w1c.rearrange("d (l h) -> d l h", l=L)
        ws1s_sb = wpks[:, 256:384]
        ws1t_sb = wpks[:, 384:512]
        idbf = wpks[:, 767:895]
        idhbf = wpks[:, 895:1023]
        fpks = wpks[:, 1024:1030].bitcast(F32)
        onesf = wpks[:, 1032:1288].bitcast(F32)
        idf32 = wpks[:, 1288:1544].bitcast(F32)
        bmean_sb = fpks[:, 0:1]
        bs1_sb = fpks[:, 1:2]
        bs2_sb = fpks[:, 2:3]

        # ---- PE pre-warm + ACT table preloads while inputs DMA in ----
        for w in range(14):
            jk = psA.tile([128, 256], F32, tag="t", name=f"jk{w}")
            nc.tensor.matmul(jk[:], z128[:, 0:128], z128[:],
                             start=True, stop=True)
        dum = workp.tile([128, 2], F32, tag="dum")
        nc.scalar.activation(dum[:], z128[:, 0:2], AF.Relu, scale=1.0)
        nc.scalar.activation(dum[:], z128[:, 0:2], AF.Sigmoid, scale=1.0)

        # ---- dummy warmup AllGather: absorbs first-collective setup ----
        warm_in = dram.tile([1, 2], BF16, tag="warmi", name="warm_in")
        warm_out = dram.tile([2, 2], BF16, tag="warmo", name="warm_out")
        nc.gpsimd.dma_start(warm_in[:], wpk[0:1, 0:2])
        nc.gpsimd.collective_compute(
            "AllGather", ALU.bypass,
            replica_groups=[[0, 1], [2, 3], [4, 5], [6, 7]],
            ins=[warm_in.opt()],
            outs=[warm_out.opt()],
        )

        # ---- encoders: tgt (512) + own-src (192), h matmuls run ahead
        # in the (free) psE pool; relus split ACT(tgt)/DVE(src) ----
        encF = psB.tile([D, N], F32, tag="acc", name="encF")
        encS = psB.tile([D, NSEND], F32, tag="acc", name="encS")
        h_f, hsb_f, h_s, hsb_s = {}, {}, {}, {}

        def h_mm(l):
            h_s[l] = psE.tile([H, NSEND], F32, tag="E", name=f"hs{l}")
            nc.tensor.matmul(h_s[l][:], w1sb[:, l, :], xsrc[:, l, :],
                             start=True, stop=True)
            h_f[l] = psE.tile([H, N], F32, tag="E", name=f"hf{l}")
            nc.tensor.matmul(h_f[l][:], w1sb[:, l, :], xfull[:, l, :],
                             start=True, stop=True)

        def relus(l):
            hsb_f[l] = workp.tile([H, N], BF16, tag="hf", name=f"hsbf{l}")
            nc.scalar.activation(hsb_f[l][:], h_f[l][:], AF.Relu,
                                 bias=b1sb[:, l:l + 1], scale=1.0)
            hsb_s[l] = workp.tile([H, NSEND], BF16, tag="hs",
                                  name=f"hsbs{l}")
            nc.vector.tensor_scalar(hsb_s[l][:], h_s[l][:],
                                    b1sb[:, l:l + 1], 0.0, ALU.add, ALU.max)

        def enc_mm(l):
            nc.tensor.matmul(encF[:], w2sb[:, l, :], hsb_f[l][:],
                             start=(l == 0), stop=(l == L - 1))
            nc.tensor.matmul(encS[:], w2sb[:, l, :], hsb_s[l][:],
                             start=(l == 0), stop=(l == L - 1))

        h_mm(0); h_mm(1); relus(0); relus(1)
        enc_mm(0); h_mm(2); relus(2); enc_mm(1)
        h_mm(3); relus(3); enc_mm(2); enc_mm(3)
        agg_f = sb.tile([D, N], BF16, tag="aggf")
        nc.scalar.activation(agg_f[:], encF[:], AF.Identity,
                             bias=bmean_sb, scale=1.0 / L)
        agg_s = sb.tile([D, NSEND], BF16, tag="aggs")
        nc.vector.tensor_scalar(agg_s[:], encS[:], 1.0 / L, bmean_sb,
                                ALU.mult, ALU.add)

        # ---- projections: tgt full; src = own 192 + fixed global
        # columns 192:256 and 448:512 of agg_f (uniform on both cores) ----
        tgt_ps = psA.tile([D, N], F32, tag="t")
        nc.tensor.matmul(tgt_ps[:], ws1t_sb, agg_f[:], start=True, stop=True)
        tgtT_bf = sb.tile([D, N], BF16, tag="tgtbf")
        nc.vector.tensor_copy(tgtT_bf[:], tgt_ps[:])
        src_ps = psB.tile([D, NSEND], F32, tag="acc")
        nc.tensor.matmul(src_ps[:], ws1s_sb, agg_s[:], start=True, stop=True)
        srcT = sb.tile([D, NS], F32, tag="srcf")
        nc.scalar.activation(srcT[:, 0:NSEND], src_ps[:], AF.Identity,
                             bias=bs1_sb, scale=1.0)
        fix_ps = psA.tile([D, 128], F32, tag="t")
        nc.tensor.matmul(fix_ps[:, 0:64], ws1s_sb, agg_f[:, 192:256],
                         start=True, stop=True)
        nc.tensor.matmul(fix_ps[:, 64:128], ws1s_sb, agg_f[:, 448:512],
                         start=True, stop=True)
        nc.scalar.activation(srcT[:, NSEND:NS], fix_ps[:], AF.Identity,
                             bias=bs1_sb, scale=1.0)
        for w in range(6):
            jk = psA.tile([128, 256], F32, tag="t", name=f"jkb{w}")
            nc.tensor.matmul(jk[:], z128[:, 0:128], z128[:],
                             start=True, stop=True)


        # ---- SBUF homes for the adjacency, its transpose, a2 ----
        Araw = [sb.tile([128, N], BF16, tag=f"Ar{kt}", name=f"Ar{kt}")
                for kt in range(NT)]
        A = [sb.tile([128, N], BF16, tag=f"A{kt}", name=f"A{kt}")
             for kt in range(NT)]
        AT = [sb.tile([128, N], BF16, tag=f"AT{kt}", name=f"AT{kt}")
              for kt in range(NT)]
        a2sb = [sb.tile([128, N], BF16, tag=f"a2{it}", name=f"a2sb{it}")
                for it in range(NT)]
        bounce = dram.tile([NSEND, N], BF16, tag="bnc", name="bounce")
        full = dram.tile([2 * NSEND, N], BF16, tag="full", name="full")
        stage = [workp.tile([64, N], BF16, tag=f"stg{i}", name=f"stg{i}")
                 for i in range(2)]

        # ---- pairwise scoring: 5 groups of 64 source slots ----
        # w2 sits at wpk column WCOL; window [WCOL-p : WCOL+64-p] puts it in
        # column p of a 64-wide lhsT -> psum row p of the (64,512) group.
        # groups 0-2 -> bounce (gather); group 3 -> Araw[1][64:], group 4 ->
        # Araw[3][64:] (via SBUF->SBUF DMA for the partition shift).
        for g in range(5):
            score_ps = psB.tile([64, N], F32, tag="acc", name=f"scps{g}")
            for p in range(64):
                s = g * 64 + p
                rt = relup.tile([D, N], BF16, tag="rt")
                if p % 16 in ACT_POS:
                    nc.scalar.activation(rt[:], tgtT_bf[:], AF.Relu,
                                         bias=srcT[:, s:s + 1], scale=1.0)
                else:
                    nc.vector.tensor_scalar(rt[:], tgtT_bf[:],
                                            srcT[:, s:s + 1], 0.0,
                                            ALU.add, ALU.max)
                nc.tensor.matmul(score_ps[:],
                                 wpks[:, WCOL - p:WCOL + 64 - p],
                                 rt[:], start=(p == 0), stop=(p == 63))
            if g <= 2:
                scbf = workp.tile([64, N], BF16, tag="scbf", name=f"scbf{g}")
                nc.scalar.activation(scbf[:], score_ps[:], AF.Sigmoid,
                                     bias=bs2_sb[0:64, :], scale=1.0)
                nc.sync.dma_start(bounce[g * 64:(g + 1) * 64, :], scbf[:])
                if g == 2:
                    nc.gpsimd.collective_compute(
                        "AllGather", ALU.bypass,
                        replica_groups=[[0, 1], [2, 3], [4, 5], [6, 7]],
                        ins=[bounce.opt()],
                        outs=[full.opt()],
                    )
            else:
                st = stage[g - 3]
                nc.scalar.activation(st[:], score_ps[:], AF.Sigmoid,
                                     bias=bs2_sb[0:64, :], scale=1.0)
                kt = 1 if g == 3 else 3
                nc.sync.dma_start(Araw[kt][64:128, :], st[:])
            if g == 3:
                # gather landed mid-G4: reload into the A k-tile homes
                nc.sync.dma_start(Araw[0][:], full[0:128, :])
                nc.sync.dma_start(Araw[1][0:64, :], full[128:192, :])
                nc.sync.dma_start(Araw[2][:], full[192:320, :])
                nc.sync.dma_start(Araw[3][0:64, :], full[320:384, :])

        # ---- threshold + zero diagonal on assembled k-tiles ----
        def finish_tile(kt):
            nc.vector.scalar_tensor_tensor(A[kt][:], Araw[kt][:], THRESH,
                                           Araw[kt][:], ALU.is_gt, ALU.mult)
            nc.gpsimd.affine_select(
                A[kt][:], A[kt][:], pattern=[[1, N]],
                compare_op=ALU.not_equal, fill=0.0,
                base=-(128 * kt), channel_multiplier=-1)

        for kt in (0, 2, 1, 3):
            finish_tile(kt)

        # ---- transposes + a2 = A@A (bf16, fp32 psum) ----
        def transpose_of(it, kt, use_act):
            pool = psA if (it + kt) % 2 == 0 else psB
            tp = pool.tile([128, 128], BF16, tag="t" if pool is psA
                           else "acc", name=f"tp{it}_{kt}")
            nc.tensor.transpose(tp[:], A[it][:, kt * 128:(kt + 1) * 128],
                                idbf)
            dst = AT[kt][:, it * 128:(it + 1) * 128]
            if use_act:
                nc.scalar.copy(dst, tp[:])
            else:
                nc.vector.tensor_copy(dst, tp[:])

        a2ps = {}

        def a2_step(it, kt, start, stop):
            if it not in a2ps:
                a2ps[it] = psE.tile([128, N], F32, tag="E",
                                    name=f"a2ps{it}")
            nc.tensor.matmul(a2ps[it][:], AT[kt][:, it * 128:(it + 1) * 128],
                             A[kt][:], start=start, stop=stop)

        for it in (0, 2, 1):
            for kt in range(NT):
                transpose_of(it, kt, (it + kt) % 2 == 0)
        for it in (0, 1, 2):
            for kt in (0, 1, 2):
                a2_step(it, kt, kt == 0, False)
        for kt in range(NT):
            transpose_of(3, kt, kt % 2 == 0)
        for it in (0, 1, 2):
            a2_step(it, 3, False, True)
            if it % 2 == 0:
                nc.vector.tensor_copy(a2sb[it][:], a2ps[it][:])
            else:
                nc.scalar.copy(a2sb[it][:], a2ps[it][:])
        for kt in range(NT):
            a2_step(3, kt, kt == 0, kt == 3)
        nc.scalar.copy(a2sb[3][:], a2ps[3][:])

        # ---- E = A + 0.5*a2 + A@a2, accumulated in PSUM ----
        E = []
        mx4 = sb.tile([128, NT], F32, tag="mx4")
        for it in range(NT):
            e_ps = psE.tile([128, N], F32, tag="E", name=f"eps{it}")
            nc.tensor.matmul(e_ps[:], idbf, A[it][:], start=True, stop=False)
            nc.tensor.matmul(e_ps[:], idhbf, a2sb[it][:], start=False,
                             stop=False)
            for kt in range(NT):
                nc.tensor.matmul(e_ps[:], AT[kt][:, it * 128:(it + 1) * 128],
                                 a2sb[kt][:], start=False, stop=(kt == 3))
            nc.vector.reduce_max(mx4[:, it:it + 1], e_ps[:],
                                 axis=mybir.AxisListType.X)
            E.append(e_ps)

        # ---- global max: per-partition max -> PE transpose to one
        # partition -> reduce -> broadcast through a ones-column matmul ----
        mxp = sb.tile([128, 1], F32, tag="mxp")
        nc.vector.reduce_max(mxp[:], mx4[:], axis=mybir.AxisListType.X)
        mxt_ps = psA.tile([1, 128], F32, tag="t", name="mxt_ps")
        nc.tensor.transpose(mxt_ps[:], mxp[:], idf32)
        mxmax = sb.tile([1, 1], F32, tag="mxmax")
        nc.vector.reduce_max(mxmax[:], mxt_ps[:], axis=mybir.AxisListType.X)
        bc_ps = psB.tile([128, 1], F32, tag="acc", name="bc_ps")
        nc.tensor.matmul(bc_ps[:], onesf[0:1, :], mxmax[:],
                         start=True, stop=True)
        denom = sb.tile([128, 1], F32, tag="denom")
        nc.vector.tensor_scalar(denom[:], bc_ps[:], 1e-8, None, ALU.add)
        recip = sb.tile([128, 1], F32, tag="recip")
        nc.vector.reciprocal(recip[:], denom[:])
        for it in range(NT):
            ot = workp.tile([128, N], BF16, tag="ot")
            if it % 2 == 0:
                nc.vector.tensor_scalar(ot[:], E[it][:], recip[:, 0:1], None,
                                        ALU.mult)
            else:
                nc.scalar.mul(ot[:], E[it][:], recip[:, 0:1])
            eng = (nc.sync, nc.scalar)[it % 2]
            eng.dma_start(outfull[it * 128:(it + 1) * 128, :], ot[:])


_NC_CACHE = {}


def _get_nc():
    if "nc" not in _NC_CACHE:
        _NC_CACHE["nc"] = _build_nc()
    return _NC_CACHE["nc"]


def _install_ntff_hook():
    try:
        from antenv.axon_hooks import get_axon_ntff_profile_hook  # noqa: F401
        return
    except ImportError:
        pass
    try:
        import importlib.util
        spec = importlib.util.spec_from_file_location(
            "trn_boot_mod", "/root/.axon_site/trn_agent_boot/trn_boot.py")
        tb = importlib.util.module_from_spec(spec)
        spec.loader.exec_module(tb)
        hook = tb._ntff_profile_via_ctypes("/opt/axon/libaxon_pjrt.so")
        m = types.ModuleType("antenv.axon_hooks")
        m.get_axon_ntff_profile_hook = lambda: hook
        m.set_axon_ntff_profile_hook = lambda h: None
        sys.modules["antenv.axon_hooks"] = m
    except Exception:
        pass


def _bf(a):
    return np.ascontiguousarray(a).astype(ml_dtypes.bfloat16)


def _prep_in_maps(x, W1, b1, W2, b2, Ws1, bs1, Ws2, bs2):
    x = np.asarray(x, np.float32)
    W1 = np.asarray(W1, np.float32)
    b1 = np.asarray(b1, np.float32)
    W2 = np.asarray(W2, np.float32)
    b2 = np.asarray(b2, np.float32)
    Ws1 = np.asarray(Ws1, np.float32)
    bs1 = np.asarray(bs1, np.float32)
    Ws2 = np.asarray(Ws2, np.float32)
    bs2 = np.asarray(bs2, np.float32)

    Tdim = x.shape[1]
    lag_idx = [max(0, Tdim - 1 - l) for l in range(L)]
    xl = x[:, lag_idx]                            # (B, L, N, D)
    xlT = np.swapaxes(xl, 2, 3)                   # (B, L, D, N)

    zwin = np.zeros((128, 255), np.float32)
    zwin[:, WCOL - 512] = Ws2[:, 0]
    fpk = np.stack([b2.mean(axis=0), bs1,
                    np.full(128, bs2[0], np.float32)], axis=1)
    fpk_bf = np.ascontiguousarray(fpk.astype(np.float32)).view(
        ml_dtypes.bfloat16)                               # (128, 6)
    wpk = np.concatenate([
        _bf(np.transpose(W1, (1, 0, 2)).reshape(D, L * H)),
        _bf(Ws1[:D]),
        _bf(Ws1[D:]),
        _bf(zwin),
        _bf(np.eye(128, dtype=np.float32)),
        _bf(0.5 * np.eye(128, dtype=np.float32)),
        np.zeros((128, 1), ml_dtypes.bfloat16),           # pad to even col
        fpk_bf,
        np.zeros((128, 2), ml_dtypes.bfloat16),
        np.ones((128, 128), np.float32).view(ml_dtypes.bfloat16),
        np.eye(128, dtype=np.float32).view(ml_dtypes.bfloat16),
    ], axis=1)                                            # (128, 1544)
    b1_bf = np.ascontiguousarray(b1.T.astype(np.float32)).view(
        ml_dtypes.bfloat16)                               # (64, 2L)
    w2pk = np.concatenate(
        [_bf(np.transpose(W2, (1, 0, 2)).reshape(H, L * D)), b1_bf], axis=1)

    common = {
        "wpk": np.ascontiguousarray(wpk),
        "w1t": np.ascontiguousarray(wpk[:, 0:256]),
        "w2r": np.ascontiguousarray(w2pk),
    }
    in_maps = []
    xdl = np.swapaxes(xlT, 1, 2)                  # (B, D, L, N)
    for c in range(NCORES):
        b, half = c // 2, c % 2
        m = dict(common)
        m["xlagT"] = _bf(xdl[b].reshape(D, L * N))
        base = half * 256
        m["xsrcP"] = _bf(xdl[b][:, :, base:base + NSEND].reshape(
            D, L * NSEND))
        in_maps.append(m)
    return in_maps


def _run(inputs, trace=False):
    nc = _get_nc()
    in_maps = _prep_in_maps(**inputs)
    if trace:
        _install_ntff_hook()
    res = run_bass_kernel_spmd(nc, in_maps, core_ids=list(range(NCORES)),
                               trace=trace)
    out = np.stack([res.results[2 * b]["outfull"] for b in range(B)],
                   axis=0).astype(np.float32)
    return out, res


def kernel(**inputs):
    out, _ = _run(inputs, trace=False)
    return out


# revision 40
# speedup vs baseline: 1.0242x; 1.0242x over previous
"""Trainium2 Bass kernel for nn_CausalPropagationAdjacency (v2).

Shapes (hardcoded): B=4, T=12, N=512, D=128, L=4, H=64.
Pipeline: lag encoders (Linear D->H, ReLU, Linear H->D, mean over L lags),
pairwise scorer sigmoid(relu(src_i+tgt_j+bs1)@Ws2+bs2), threshold 0.1, zero
diagonal, enhanced = A + 0.5 A^2 + 0.25 A^3, normalize by per-batch max.

Sharding: 8 cores = 4 batch-pairs. Each core scores 320 source rows: its
own 256 plus a REDUNDANT copy of the 64 rows of the peer's 4th group, so
only ONE AllGather (of each core's first 192 rows) is needed and its
~12-30us latency hides under the scoring of groups 3-4. Gather output is
rank-ordered == global row order, so every core assembles the full
(512,512) adjacency identically. Pure SPMD: the only per-core input
difference is xsrcP (the core's own 192 rows' lag slices); the two shared
64-row blocks (global rows 192:256 and 448:512) are projected from the
full target encoding with compile-time slices on BOTH cores. Host reads
core 2b's output.

Scoring: per source row one DVE tensor_scalar (add+max0) or ACT
Relu-with-bias produces relu(src_i+tgt+bs1) as a (128,512) bf16 tile; a
matmul against a 64-wide sliding window of the packed weight buffer (w2
embedded in one column) accumulates row i%64 of a (64,512) score block in
PSUM at the full 216ns/row streaming rate. 5 groups of 64 rows; groups 0-2
are sent (sigmoid->bf16->DMA->AllGather), groups 3/4 fill the two locally
known 64-row blocks via SBUF->SBUF DMA (partition shift). Threshold +
diagonal-zero run once per assembled A k-tile (off the scoring critical
path; gpsimd affine_select). Hops (transposes, A^2, A@a2,
identity-accumulated E) all in bf16 (fp32 PSUM accumulate). Global max:
per-partition max -> PE fp32 transpose -> one-partition reduce -> ones
matmul broadcast (avoids the gpsimd partition_all_reduce ucode-library
swap, ~10us). PE is pre-warmed with junk matmuls during the input-DMA
wait and between encoder and scoring so HAM stays at K=8/8; dummy ACT ops
preload both activation table sets off the critical path.
"""

import sys
import types
import numpy as np
import ml_dtypes

import concourse.bacc as bacc
import concourse.bass as bass
import concourse.bass_isa as bass_isa
import concourse.mybir as mybir
import concourse.tile as tile
from concourse.bass_utils import run_bass_kernel_spmd

B, T, N, D = 4, 12, 512, 128
L, H = 4, 64
THRESH = 0.1
NCORES = 8
NS = 320          # source slots per core: own 192 + block@192 + block@448
NSEND = 192       # rows sent through the AllGather
NT = N // 128
F32 = mybir.dt.float32
BF16 = mybir.dt.bfloat16
AF = mybir.ActivationFunctionType
ALU = mybir.AluOpType

# pairwise engine assignment per p%16 (DVE ~330-350ns/tile, ACT ~700ns/tile)
ACT_POS = {1, 4, 7, 10, 13}

WCOL = 639        # absolute wpk column holding Ws2 (inside the zero window)


def _build_nc():
    nc = bacc.Bacc("TRN2", target_bir_lowering=False, debug=False,
                   num_devices=NCORES)
    # host pre-transposed to (D, L*n) so the input DMAs are contiguous
    xlagT = nc.dram_tensor("xlagT", [D, L * N], BF16, kind="ExternalInput")
    # this core's own first 192 rows' lag slices (the per-core part)
    xsrcP = nc.dram_tensor("xsrcP", [D, L * NSEND], BF16,
                           kind="ExternalInput")
    # packed bf16 weights: [w1r(L*H=256) | ws1s(128) | ws1t(128) | zwin(255,
    #   w2 at abs col 639) | idbf(128) | 0.5*idbf(128) | pad(1)
    #   | bitcast f32 [bmean|bs1|bs2] (6) | pad(2)]
    wpk = nc.dram_tensor("wpk", [128, 1544], BF16, kind="ExternalInput")
    # w2r (64, L*D) bf16 + b1 (64, L) f32 bitcast to 2*L bf16 cols
    w2r = nc.dram_tensor("w2r", [H, L * D + 2 * L], BF16,
                         kind="ExternalInput")
    # bf16 output (host upcasts): halves the final DMA and speeds the
    # PSUM-read scale ops; ~4e-3 extra error vs the 2e-2 gate
    outfull = nc.dram_tensor("outfull", [N, N], BF16, kind="ExternalOutput")

    with tile.TileContext(nc) as tc:
        _emit(nc, tc, xlagT, xsrcP, wpk, w2r, outfull)
    nc.compile()
    return nc


def _emit(nc, tc, xlagT, xsrcP, wpk, w2r, outfull):
    from contextlib import ExitStack
    ctx = ExitStack()
    with ctx:
        consts = ctx.enter_context(tc.tile_pool(name="consts", bufs=1))
        sb = ctx.enter_context(tc.tile_pool(name="sb", bufs=1))
        relup = ctx.enter_context(tc.tile_pool(name="relu", bufs=10))
        workp = ctx.enter_context(tc.tile_pool(name="work", bufs=4))
        psA = ctx.enter_context(tc.tile_pool(name="psA", bufs=2, space="PSUM"))
        psB = ctx.enter_context(tc.tile_pool(name="psB", bufs=2, space="PSUM"))
        psE = ctx.enter_context(tc.tile_pool(name="psE", bufs=4, space="PSUM"))
        dram = ctx.enter_context(tc.tile_pool(name="dram", bufs=1,
                                              space="DRAM"))

        # ---- zero tile for PE pre-warm (no input dependency) ----
        z128 = consts.tile([128, 256], BF16, tag="z128")
        nc.gpsimd.memset(z128[:], 0.0)


        # ---- input DMAs: sync queue (small first); xlagT on scalar queue ----
        xfull_ = consts.tile([D, L * N], BF16, tag="xf")
        nc.scalar.dma_start(xfull_[:], xlagT[:])
        wpks = consts.tile([128, 1544], BF16, tag="wpk")
        nc.sync.dma_start(wpks[:], wpk[:])
        xsrc_ = consts.tile([D, L * NSEND], BF16, tag="xs")
        nc.sync.dma_start(xsrc_[:], xsrcP[:])
        w2pk = consts.tile([H, L * D + 2 * L], BF16, tag="w2")
        nc.sync.dma_start(w2pk[:], w2r[:])
        xfull = xfull_.rearrange("d (l n) -> d l n", l=L)
        xsrc = xsrc_.rearrange("d (l n) -> d l n", l=L)

        w2sb = w2pk[:, 0:L * D].rearrange("h (l d) -> h l d", l=L)
        b1sb = w2pk[:, L * D:L * D + 2 * L].bitcast(F32)
        w1sb = wpks[:, 0:256].rearrange("d (l h) -> d l h", l=L)
        ws1s_sb = wpks[:, 256:384]
        ws1t_sb = wpks[:, 384:512]
        idbf = wpks[:, 767:895]
        idhbf = wpks[:, 895:1023]
        fpks = wpks[:, 1024:1030].bitcast(F32)
        onesf = wpks[:, 1032:1288].bitcast(F32)
        idf32 = wpks[:, 1288:1544].bitcast(F32)
        bmean_sb = fpks[:, 0:1]
        bs1_sb = fpks[:, 1:2]
        bs2_sb = fpks[:, 2:3]

        # ---- PE pre-warm + ACT table preloads while inputs DMA in ----
        for w in range(24):
            jk = psA.tile([128, 256], F32, tag="t", name=f"jk{w}")
            nc.tensor.matmul(jk[:], z128[:, 0:128], z128[:],
                             start=True, stop=True)
        dum = workp.tile([128, 2], F32, tag="dum")
        nc.scalar.activation(dum[:], z128[:, 0:2], AF.Relu, scale=1.0)
        nc.scalar.activation(dum[:], z128[:, 0:2], AF.Sigmoid, scale=1.0)

        # ---- dummy warmup AllGather: absorbs first-collective setup ----
        warm_in = dram.tile([1, 2], BF16, tag="warmi", name="warm_in")
        warm_out = dram.tile([2, 2], BF16, tag="warmo", name="warm_out")
        nc.gpsimd.dma_start(warm_in[:], wpk[0:1, 0:2])
        nc.gpsimd.collective_compute(
            "AllGather", ALU.bypass,
            replica_groups=[[0, 1], [2, 3], [4, 5], [6, 7]],
            ins=[warm_in.opt()],
            outs=[warm_out.opt()],
        )

        # ---- encoders: tgt (512) + own-src (192), h matmuls run ahead
        # in the (free) psE pool; relus split ACT(tgt)/DVE(src) ----
        encF = psB.tile([D, N], F32, tag="acc", name="encF")
        encS = psB.tile([D, NSEND], F32, tag="acc", name="encS")
        h_f, hsb_f, h_s, hsb_s = {}, {}, {}, {}

        def h_mm(l):
            h_f[l] = psE.tile([H, N], F32, tag="E", name=f"hf{l}")
            nc.tensor.matmul(h_f[l][:], w1sb[:, l, :], xfull[:, l, :],
                             start=True, stop=True)
            h_s[l] = psE.tile([H, NSEND], F32, tag="E", name=f"hs{l}")
            nc.tensor.matmul(h_s[l][:], w1sb[:, l, :], xsrc[:, l, :],
                             start=True, stop=True)

        def relus(l):
            hsb_f[l] = workp.tile([H, N], BF16, tag="hf", name=f"hsbf{l}")
            nc.scalar.activation(hsb_f[l][:], h_f[l][:], AF.Relu,
                                 bias=b1sb[:, l:l + 1], scale=1.0)
            hsb_s[l] = workp.tile([H, NSEND], BF16, tag="hs",
                                  name=f"hsbs{l}")
            nc.vector.tensor_scalar(hsb_s[l][:], h_s[l][:],
                                    b1sb[:, l:l + 1], 0.0, ALU.add, ALU.max)

        def enc_mm(l):
            nc.tensor.matmul(encF[:], w2sb[:, l, :], hsb_f[l][:],
                             start=(l == 0), stop=(l == L - 1))
            nc.tensor.matmul(encS[:], w2sb[:, l, :], hsb_s[l][:],
                             start=(l == 0), stop=(l == L - 1))

        h_mm(0); h_mm(1); relus(0); relus(1)
        enc_mm(0); h_mm(2); relus(2); enc_mm(1)
        h_mm(3); relus(3); enc_mm(2); enc_mm(3)
        agg_f = sb.tile([D, N], BF16, tag="aggf")
        nc.scalar.activation(agg_f[:], encF[:], AF.Identity,
                             bias=bmean_sb, scale=1.0 / L)
        agg_s = sb.tile([D, NSEND], BF16, tag="aggs")
        nc.vector.tensor_scalar(agg_s[:], encS[:], 1.0 / L, bmean_sb,
                                ALU.mult, ALU.add)

        # ---- projections: tgt full; src = own 192 + fixed global
        # columns 192:256 and 448:512 of agg_f (uniform on both cores) ----
        tgt_ps = psA.tile([D, N], F32, tag="t")
        nc.tensor.matmul(tgt_ps[:], ws1t_sb, agg_f[:], start=True, stop=True)
        tgtT_bf = sb.tile([D, N], BF16, tag="tgtbf")
        nc.vector.tensor_copy(tgtT_bf[:], tgt_ps[:])
        src_ps = psB.tile([D, NSEND], F32, tag="acc")
        nc.tensor.matmul(src_ps[:], ws1s_sb, agg_s[:], start=True, stop=True)
        srcT = sb.tile([D, NS], F32, tag="srcf")
        nc.scalar.activation(srcT[:, 0:NSEND], src_ps[:], AF.Identity,
                             bias=bs1_sb, scale=1.0)
        fix_ps = psA.tile([D, 128], F32, tag="t")
        nc.tensor.matmul(fix_ps[:, 0:64], ws1s_sb, agg_f[:, 192:256],
                         start=True, stop=True)
        nc.tensor.matmul(fix_ps[:, 64:128], ws1s_sb, agg_f[:, 448:512],
                         start=True, stop=True)
        nc.scalar.activation(srcT[:, NSEND:NS], fix_ps[:], AF.Identity,
                             bias=bs1_sb, scale=1.0)
        for w in range(6):
            jk = psA.tile([128, 256], F32, tag="t", name=f"jkb{w}")
            nc.tensor.matmul(jk[:], z128[:, 0:128], z128[:],
                             start=True, stop=True)


        # ---- SBUF homes for the adjacency, its transpose, a2 ----
        Araw = [sb.tile([128, N], BF16, tag=f"Ar{kt}", name=f"Ar{kt}")
                for kt in range(NT)]
        A = [sb.tile([128, N], BF16, tag=f"A{kt}", name=f"A{kt}")
             for kt in range(NT)]
        AT = [sb.tile([128, N], BF16, tag=f"AT{kt}", name=f"AT{kt}")
              for kt in range(NT)]
        a2sb = [sb.tile([128, N], BF16, tag=f"a2{it}", name=f"a2sb{it}")
                for it in range(NT)]
        bounce = dram.tile([NSEND, N], BF16, tag="bnc", name="bounce")
        full = dram.tile([2 * NSEND, N], BF16, tag="full", name="full")
        stage = [workp.tile([64, N], BF16, tag=f"stg{i}", name=f"stg{i}")
                 for i in range(2)]

        # ---- pairwise scoring: 5 groups of 64 source slots ----
        # w2 sits at wpk column WCOL; window [WCOL-p : WCOL+64-p] puts it in
        # column p of a 64-wide lhsT -> psum row p of the (64,512) group.
        # groups 0-2 -> bounce (gather); group 3 -> Araw[1][64:], group 4 ->
        # Araw[3][64:] (via SBUF->SBUF DMA for the partition shift).
        for g in range(5):
            score_ps = psB.tile([64, N], F32, tag="acc", name=f"scps{g}")
            for p in range(64):
                s = g * 64 + p
                rt = relup.tile([D, N], BF16, tag="rt")
                if p % 16 in ACT_POS:
                    nc.scalar.activation(rt[:], tgtT_bf[:], AF.Relu,
                                         bias=srcT[:, s:s + 1], scale=1.0)
                else:
                    nc.vector.tensor_scalar(rt[:], tgtT_bf[:],
                                            srcT[:, s:s + 1], 0.0,
                                            ALU.add, ALU.max)
                nc.tensor.matmul(score_ps[:],
                                 wpks[:, WCOL - p:WCOL + 64 - p],
                                 rt[:], start=(p == 0), stop=(p == 63))
            if g <= 2:
                scbf = workp.tile([64, N], BF16, tag="scbf", name=f"scbf{g}")
                nc.scalar.activation(scbf[:], score_ps[:], AF.Sigmoid,
                                     bias=bs2_sb[0:64, :], scale=1.0)
                nc.sync.dma_start(bounce[g * 64:(g + 1) * 64, :], scbf[:])
                if g == 2:
                    nc.gpsimd.collective_compute(
                        "AllGather", ALU.bypass,
                        replica_groups=[[0, 1], [2, 3], [4, 5], [6, 7]],
                        ins=[bounce.opt()],
                        outs=[full.opt()],
                    )
            else:
                st = stage[g - 3]
                nc.scalar.activation(st[:], score_ps[:], AF.Sigmoid,
                                     bias=bs2_sb[0:64, :], scale=1.0)
                kt = 1 if g == 3 else 3
                nc.sync.dma_start(Araw[kt][64:128, :], st[:])
            if g == 3:
                # gather landed mid-G4: reload into the A k-tile homes
                nc.sync.dma_start(Araw[0][:], full[0:128, :])
                nc.sync.dma_start(Araw[1][0:64, :], full[128:192, :])
                nc.sync.dma_start(Araw[2][:], full[192:320, :])
                nc.sync.dma_start(Araw[3][0:64, :], full[320:384, :])

        # ---- threshold + zero diagonal on assembled k-tiles ----
        def finish_tile(kt):
            nc.vector.scalar_tensor_tensor(A[kt][:], Araw[kt][:], THRESH,
                                           Araw[kt][:], ALU.is_gt, ALU.mult)
            nc.gpsimd.affine_select(
                A[kt][:], A[kt][:], pattern=[[1, N]],
                compare_op=ALU.not_equal, fill=0.0,
                base=-(128 * kt), channel_multiplier=-1)

        for kt in (0, 2, 1, 3):
            finish_tile(kt)

        # ---- transposes + a2 = A@A (bf16, fp32 psum) ----
        def transpose_of(it, kt, use_act):
            pool = psA if (it + kt) % 2 == 0 else psB
            tp = pool.tile([128, 128], BF16, tag="t" if pool is psA
                           else "acc", name=f"tp{it}_{kt}")
            nc.tensor.transpose(tp[:], A[it][:, kt * 128:(kt + 1) * 128],
                                idbf)
            dst = AT[kt][:, it * 128:(it + 1) * 128]
            if use_act:
                nc.scalar.copy(dst, tp[:])
            else:
                nc.vector.tensor_copy(dst, tp[:])

        a2ps = {}

        def a2_step(it, kt, start, stop):
            if it not in a2ps:
                a2ps[it] = psE.tile([128, N], F32, tag="E",
                                    name=f"a2ps{it}")
            nc.tensor.matmul(a2ps[it][:], AT[kt][:, it * 128:(it + 1) * 128],
                             A[kt][:], start=start, stop=stop)

        for it in (0, 2, 1):
            for kt in range(NT):
                transpose_of(it, kt, (it + kt) % 2 == 0)
        for it in (0, 1, 2):
            for kt in (0, 1, 2):
                a2_step(it, kt, kt == 0, False)
        for kt in range(NT):
            transpose_of(3, kt, kt % 2 == 0)
        for it in (0, 1, 2):
            a2_step(it, 3, False, True)
            if it % 2 == 0:
                nc.vector.tensor_copy(a2sb[it][:], a2ps[it][:])
            else:
                nc.scalar.copy(a2sb[it][:], a2ps[it][:])
        for kt in range(NT):
            a2_step(3, kt, kt == 0, kt == 3)
        nc.scalar.copy(a2sb[3][:], a2ps[3][:])

        # ---- E = A + 0.5*a2 + A@a2, accumulated in PSUM ----
        E = []
        mx4 = sb.tile([128, NT], F32, tag="mx4")
        for it in range(NT):
            e_ps = psE.tile([128, N], F32, tag="E", name=f"eps{it}")
            nc.tensor.matmul(e_ps[:], idbf, A[it][:], start=True, stop=False)
            nc.tensor.matmul(e_ps[:], idhbf, a2sb[it][:], start=False,
                             stop=False)
            for kt in range(NT):
                nc.tensor.matmul(e_ps[:], AT[kt][:, it * 128:(it + 1) * 128],
                                 a2sb[kt][:], start=False, stop=(kt == 3))
            nc.vector.reduce_max(mx4[:, it:it + 1], e_ps[:],
                                 axis=mybir.AxisListType.X)
            E.append(e_ps)

        # ---- global max: per-partition max -> PE transpose to one
        # partition -> reduce -> broadcast through a ones-column matmul ----
        mxp = sb.tile([128, 1], F32, tag="mxp")
        nc.vector.reduce_max(mxp[:], mx4[:], axis=mybir.AxisListType.X)
        mxt_ps = psA.tile([1, 128], F32, tag="t", name="mxt_ps")
        nc.tensor.transpose(mxt_ps[:], mxp[:], idf32)
        mxmax = sb.tile([1, 1], F32, tag="mxmax")
        nc.vector.reduce_max(mxmax[:], mxt_ps[:], axis=mybir.AxisListType.X)
        bc_ps = psB.tile([128, 1], F32, tag="acc", name="bc_ps")
        nc.tensor.matmul(bc_ps[:], onesf[0:1, :], mxmax[:],
                         start=True, stop=True)
        denom = sb.tile([128, 1], F32, tag="denom")
        nc.vector.tensor_scalar(denom[:], bc_ps[:], 1e-8, None, ALU.add)
        recip = sb.tile([128, 1], F32, tag="recip")
        nc.vector.reciprocal(recip[:], denom[:])
        for it in range(NT):
            ot = workp.tile([128, N], BF16, tag="ot")
            if it % 2 == 0:
                nc.vector.tensor_scalar(ot[:], E[it][:], recip[:, 0:1], None,
                                        ALU.mult)
            else:
                nc.scalar.mul(ot[:], E[it][:], recip[:, 0:1])
            eng = (nc.sync, nc.scalar)[it % 2]
            eng.dma_start(outfull[it * 128:(it + 1) * 128, :], ot[:])


_NC_CACHE = {}


def _get_nc():
    if "nc" not in _NC_CACHE:
        _NC_CACHE["nc"] = _build_nc()
    return _NC_CACHE["nc"]


def _install_ntff_hook():
    try:
        from antenv.axon_hooks import get_axon_ntff_profile_hook  # noqa: F401
        return
    except ImportError:
        pass
    try:
        import importlib.util
        spec = importlib.util.spec_from_file_location(
            "trn_boot_mod", "/root/.axon_site/trn_agent_boot/trn_boot.py")
        tb = importlib.util.module_from_spec(spec)
        spec.loader.exec_module(tb)
        hook = tb._ntff_profile_via_ctypes("/opt/axon/libaxon_pjrt.so")
        m = types.ModuleType("antenv.axon_hooks")
        m.get_axon_ntff_profile_hook = lambda: hook
        m.set_axon_ntff_profile_hook = lambda h: None
        sys.modules["antenv.axon_hooks"] = m
    except Exception:
        pass


def _bf(a):
    return np.ascontiguousarray(a).astype(ml_dtypes.bfloat16)


def _prep_in_maps(x, W1, b1, W2, b2, Ws1, bs1, Ws2, bs2):
    x = np.asarray(x, np.float32)
    W1 = np.asarray(W1, np.float32)
    b1 = np.asarray(b1, np.float32)
    W2 = np.asarray(W2, np.float32)
    b2 = np.asarray(b2, np.float32)
    Ws1 = np.asarray(Ws1, np.float32)
    bs1 = np.asarray(bs1, np.float32)
    Ws2 = np.asarray(Ws2, np.float32)
    bs2 = np.asarray(bs2, np.float32)

    Tdim = x.shape[1]
    lag_idx = [max(0, Tdim - 1 - l) for l in range(L)]
    xl = x[:, lag_idx]                            # (B, L, N, D)
    xlT = np.swapaxes(xl, 2, 3)                   # (B, L, D, N)

    zwin = np.zeros((128, 255), np.float32)
    zwin[:, WCOL - 512] = Ws2[:, 0]
    fpk = np.stack([b2.mean(axis=0), bs1,
                    np.full(128, bs2[0], np.float32)], axis=1)
    fpk_bf = np.ascontiguousarray(fpk.astype(np.float32)).view(
        ml_dtypes.bfloat16)                               # (128, 6)
    wpk = np.concatenate([
        _bf(np.transpose(W1, (1, 0, 2)).reshape(D, L * H)),
        _bf(Ws1[:D]),
        _bf(Ws1[D:]),
        _bf(zwin),
        _bf(np.eye(128, dtype=np.float32)),
        _bf(0.5 * np.eye(128, dtype=np.float32)),
        np.zeros((128, 1), ml_dtypes.bfloat16),           # pad to even col
        fpk_bf,
        np.zeros((128, 2), ml_dtypes.bfloat16),
        np.ones((128, 128), np.float32).view(ml_dtypes.bfloat16),
        np.eye(128, dtype=np.float32).view(ml_dtypes.bfloat16),
    ], axis=1)                                            # (128, 1544)
    b1_bf = np.ascontiguousarray(b1.T.astype(np.float32)).view(
        ml_dtypes.bfloat16)                               # (64, 2L)
    w2pk = np.concatenate(
        [_bf(np.transpose(W2, (1, 0, 2)).reshape(H, L * D)), b1_bf], axis=1)

    common = {
        "wpk": np.ascontiguousarray(wpk),
        "w2r": np.ascontiguousarray(w2pk),
    }
    in_maps = []
    xdl = np.swapaxes(xlT, 1, 2)                  # (B, D, L, N)
    for c in range(NCORES):
        b, half = c // 2, c % 2
        m = dict(common)
        m["xlagT"] = _bf(xdl[b].reshape(D, L * N))
        base = half * 256
        m["xsrcP"] = _bf(xdl[b][:, :, base:base + NSEND].reshape(
            D, L * NSEND))
        in_maps.append(m)
    return in_maps


def _run(inputs, trace=False):
    nc = _get_nc()
    in_maps = _prep_in_maps(**inputs)
    if trace:
        _install_ntff_hook()
    res = run_bass_kernel_spmd(nc, in_maps, core_ids=list(range(NCORES)),
                               trace=trace)
    out = np.stack([res.results[2 * b]["outfull"] for b in range(B)],
                   axis=0).astype(np.float32)
    return out, res


def kernel(**inputs):
    out, _ = _run(inputs, trace=False)
    return out
